# revision 54
# baseline (speedup 1.0000x reference)
"""Multi-head attention (B=2, S=2048, D=1024, H=16) on 8 Trainium2 cores.

Sharding: tensor-parallel over heads — each core owns 2 heads (a 128-feature
slice) for both batches.  Per core:
  - QKV projections for its feature slice (full tokens), transposed layout
  - causal attention for its 4 (batch, head) pairs with block-skipping
  - partial output projection (contraction over its 128 features)
Host: transposes/prepares inputs, sums the 8 partial outputs, adds bo.

On-chip pipeline: inputs/weights arrive in IN_DT (bf16 default), q/k
projections are stored float32r and the score matmuls run float32r (full PE
rate at moving-dim >= 256, near-fp32 precision); the v path and attention
probabilities are bf16 (PV runs bf16 at full rate).  v is transposed to
token-major with small PE transposes.  The causal mask is applied with
GPSIMD affine_select on the exp output — masked tiles' PV matmuls are
deferred to the end of each block so the in-order PE never waits on the
(slow) mask op, and fully-masked column prefixes of diagonal tiles are
skipped in scores/exp/PV.  Cross-phase work (next-batch projections,
finished-block output projections) is interleaved into the attention
instruction stream with DMA-rate pacing.
"""

import os

import numpy as np
import ml_dtypes

D_MODEL = 1024
NUM_HEADS = 16
DEPTH = 64
BATCH = 2
SEQ = 2048
NTOK = BATCH * SEQ  # 4096
N_CORES = 8
FW = 128  # features per core (2 heads x 64)
P = 128
SB = 512  # s-block width
N_SB = SEQ // SB  # 4 s-blocks per batch
N_TT = SEQ // P  # 16 t-tiles per batch
N_NB = NTOK // SB  # 8 n-blocks over all tokens
N_CT = D_MODEL // P  # 8 contraction tiles

# bf16 inputs halve DMA traffic; fp32 inputs maximize accuracy.
IN_BF16 = os.environ.get("MHA_IN_BF16", "1") == "1"

LAST_RESULTS = None  # BassKernelResults from the most recent kernel() call
LAST_EXEC_WALL = None  # wall seconds of the run_bass_kernel_spmd call


def _mask_structure(mask_np):
    """Classify each (t-tile, s-block) of the [S, S] mask (1.0 = disallowed).

    Returns (kind, mix_idx, patterns): kind[i][j] in
    {"skip", "plain", "affine", "mixed"}; for "affine", mix_idx[i][j] is the
    offset c of keep = (s >= c + t); for "mixed" it indexes into patterns
    (list of [P, SB] keep-masks).  mask rows = query s, cols = key t;
    scoresT is [t, s] so we transpose.
    """
    maskT = np.ascontiguousarray(mask_np.reshape(SEQ, SEQ).T)
    kind = [[None] * N_SB for _ in range(N_TT)]
    mix_idx = [[None] * N_SB for _ in range(N_TT)]
    patterns = []
    pat_key = {}
    s_idx = np.arange(SB)[None, :]
    t_idx = np.arange(P)[:, None]
    for i in range(N_TT):
        for j in range(N_SB):
            sub = maskT[i * P : (i + 1) * P, j * SB : (j + 1) * SB]
            if np.all(sub >= 0.5):
                kind[i][j] = "skip"
                continue
            if np.all(sub < 0.5):
                kind[i][j] = "plain"
                continue
            keep = (sub < 0.5).astype(np.float32)
            first_one = np.argmax(keep, axis=1)
            c = int(first_one[0])
            if np.array_equal(keep, (s_idx >= c + t_idx).astype(np.float32)):
                kind[i][j] = "affine"
                mix_idx[i][j] = c
                continue
            kind[i][j] = "mixed"
            key = keep.tobytes()
            if key not in pat_key:
                pat_key[key] = len(patterns)
                patterns.append(keep)
            mix_idx[i][j] = pat_key[key]
    return kind, mix_idx, patterns


def _build_nc(kind, mix_idx, n_patterns, in_bf16, has_bias):
    import concourse.tile as tile
    import concourse.mybir as mybir
    from concourse import bacc

    dt = mybir.dt
    # walrus requires every producer feeding an fp32r matmul to emit
    # fp32r-typed (rounded) values, so fp32-mode inputs and the q/k/ao/wo
    # activations are declared float32r outright (same bytes as fp32).
    IN_DT = dt.bfloat16 if in_bf16 else dt.float32r

    nc = bacc.Bacc(None, target_bir_lowering=False)

    qT = nc.dram_tensor("qT", [D_MODEL, NTOK], IN_DT, kind="ExternalInput")
    kT = nc.dram_tensor("kT", [D_MODEL, NTOK], IN_DT, kind="ExternalInput")
    vT = nc.dram_tensor("vT", [D_MODEL, NTOK], IN_DT, kind="ExternalInput")
    wq = nc.dram_tensor("wqT", [D_MODEL, FW], IN_DT, kind="ExternalInput")
    wk = nc.dram_tensor("wkT", [D_MODEL, FW], IN_DT, kind="ExternalInput")
    wv = nc.dram_tensor("wvT", [D_MODEL, FW], IN_DT, kind="ExternalInput")
    wo = nc.dram_tensor("woT", [FW, D_MODEL], dt.float32r, kind="ExternalInput")
    maskt = None
    if n_patterns:
        maskt = nc.dram_tensor(
            "maskt", [n_patterns, P, SB], dt.bfloat16, kind="ExternalInput"
        )
    bq = bk = bv = None
    if has_bias:
        bq = nc.dram_tensor("bq", [FW, 1], dt.float32, kind="ExternalInput")
        bk = nc.dram_tensor("bk", [FW, 1], dt.float32, kind="ExternalInput")
        bv = nc.dram_tensor("bv", [FW, 1], dt.float32, kind="ExternalInput")
    identT = nc.dram_tensor("identT", [P, 64], dt.bfloat16, kind="ExternalInput")
    outT = nc.dram_tensor("outT", [D_MODEL, NTOK], dt.bfloat16, kind="ExternalOutput")

    with tile.TileContext(nc) as tc:
        with (
            tc.tile_pool(name="const", bufs=1) as const,
            tc.tile_pool(name="big", bufs=1) as big,
            tc.tile_pool(name="stageA", bufs=5) as stA,
            tc.tile_pool(name="expp", bufs=6) as expp,
            tc.tile_pool(name="linp", bufs=2) as linp,
            tc.tile_pool(name="outst", bufs=4) as outst,
            # PSUM: sc 2x2 banks + pv 2x1 + shared proj/outproj 2 = 8 banks
            tc.tile_pool(name="mmps", bufs=2, space="PSUM") as mmps,
            tc.tile_pool(name="scps", bufs=2, space="PSUM") as scps,
            tc.tile_pool(name="pvps", bufs=1, space="PSUM") as pvps,
        ):
            # ---- constants ----
            wq_sb = const.tile([P, N_CT, FW], IN_DT)
            wk_sb = const.tile([P, N_CT, FW], IN_DT)
            wv_sb = const.tile([P, N_CT, FW], IN_DT)
            nc.sync.dma_start(wq_sb, wq.rearrange("(ct p) f -> p ct f", p=P))
            nc.sync.dma_start(wk_sb, wk.rearrange("(ct p) f -> p ct f", p=P))
            nc.sync.dma_start(wv_sb, wv.rearrange("(ct p) f -> p ct f", p=P))
            wo_sb = const.tile([P, N_CT, P], dt.float32r)
            nc.sync.dma_start(wo_sb, wo.rearrange("f (dt p) -> f dt p", p=P))
            mask_sb = None
            if n_patterns:
                mask_sb = const.tile([P, n_patterns, SB], dt.bfloat16)
                nc.sync.dma_start(mask_sb, maskt.rearrange("m p s -> p m s"))
            # [128, 64] = eye(64) stacked twice (bf16): PE-transpose identity;
            # the h=1 slice needs base partition 64.
            ident = const.tile([P, 64], dt.bfloat16)
            nc.sync.dma_start(ident, identT[:, :])
            bias_sb = {}
            if has_bias:
                for name, t in (("q", bq), ("k", bk), ("v", bv)):
                    bias_sb[name] = const.tile([P, 1], dt.float32)
                    nc.sync.dma_start(bias_sb[name], t[:, :])

            # ---- persistent activations ----
            qh_sb = big.tile([P, NTOK], dt.float32r)
            kh_sb = big.tile([P, NTOK], dt.float32r)
            ao_sb = big.tile([P, NTOK], dt.float32r)
            vhT_sb = big.tile([P, NTOK], dt.bfloat16)
            # per (b, h): [t', t-tile, 128] bf16; one 64-col half holds v
            # (written by PE transposes), the other is 1.0 so the PV
            # matmul also produces the softmax row-sums l:
            #   h0: lhsT = [v | 1] -> psum [data(0:64); l(64:128)]
            #   h1: lhsT = [1 | v] -> psum [l(0:64); data(64:128)]
            vh_sb = [
                big.tile([P, N_TT, P], dt.bfloat16, name=f"vh{pair}")
                for pair in range(4)
            ]
            for pair in range(4):
                h = pair % 2
                ones_sl = slice(64, 128) if h == 0 else slice(0, 64)
                nc.vector.memset(vh_sb[pair][:, :, ones_sl], 1.0)

            def proj_copyback(dst_ap, ps, bias_tile):
                if bias_tile is not None:
                    nc.vector.tensor_tensor(
                        dst_ap, ps, bias_tile.to_broadcast(ps.shape),
                        mybir.AluOpType.add,
                    )
                else:
                    nc.vector.tensor_copy(dst_ap, ps)

            def project_nb(src, w_sb, dst, bn, nb):
                """Project one 512-token block of one of q/k/vT."""
                st = stA.tile([P, N_CT, SB], IN_DT, tag="st", name="st")
                nc.sync.dma_start(
                    st,
                    src[:, nb * SB : (nb + 1) * SB].rearrange(
                        "(ct p) n -> p ct n", p=P
                    ),
                )
                ps = mmps.tile([P, SB], dt.float32, tag="ps", name="ps")
                for ct in range(N_CT):
                    nc.tensor.matmul(
                        ps,
                        lhsT=w_sb[:, ct, :],
                        rhs=st[:, ct, :],
                        start=(ct == 0),
                        stop=(ct == N_CT - 1),
                    )
                proj_copyback(
                    dst[:, nb * SB : (nb + 1) * SB],
                    ps,
                    bias_sb.get(bn) if has_bias else None,
                )

            def transpose_vh(b, h, m):
                """One 512-token chunk of vhT [64, SB] -> vh natural
                [128, 4, 64] via PE transposes (the XBAR DMA transpose
                serializes against every in-flight DMA on mode switch)."""
                pair = 2 * b + h
                data_sl = slice(0, 64) if h == 0 else slice(64, 128)
                hsl = slice(h * 64, h * 64 + 64)
                for tt in range(4 * m, 4 * m + 4):
                    tok0 = b * SEQ + tt * P
                    pst = mmps.tile([P, 64], dt.bfloat16, tag="ps", name="tp")
                    nc.tensor.transpose(
                        pst, vhT_sb[hsl, tok0 : tok0 + P], ident[hsl, :]
                    )
                    nc.vector.tensor_copy(vh_sb[pair][:, tt, data_sl], pst)

            def flush_filler_upto(b, j):
                """Emit all queued units that attention block (b, j) depends
                on (its own batch's projections/transposes up to block j)."""
                while filler and (
                    filler[0][2] is None or filler[0][2] <= (b, j)
                ):
                    unit, cost, _ = filler.popleft()
                    unit()

            def attention_block(b, j):
                flush_filler_upto(b, j)
                ilist = [i for i in range(N_TT) if kind[i][j] != "skip"]
                assert ilist, "fully-masked s-block unsupported"

                pv = [
                    pvps.tile([P, SB], dt.float32, tag=f"pv{h}", name=f"pv{h}")
                    for h in range(2)
                ]
                ssl = slice(b * SEQ + j * SB, b * SEQ + (j + 1) * SB)
                n_pv = len(ilist)
                pv_emitted = 0
                deferred = []

                def emit_pv(i, e, s0):
                    nonlocal pv_emitted
                    for h in range(2):
                        nc.tensor.matmul(
                            pv[h][:, s0:],
                            lhsT=vh_sb[2 * b + h][:, i, :],
                            rhs=e[:, h, s0:],
                            start=(pv_emitted == 0),
                            stop=(pv_emitted == n_pv - 1),
                        )
                    pv_emitted += 1

                # an affine tile with offset c has columns s' < c fully
                # masked: skip them in scores/exp/PV. Only safe if the
                # first-emitted PV (which clears the psum accumulation
                # group) covers the whole block.
                can_trim = any(
                    kind[i][j] == "plain"
                    or (kind[i][j] == "affine" and mix_idx[i][j] == 0)
                    for i in ilist
                )

                def trim(i):
                    if can_trim and kind[i][j] == "affine":
                        return min(mix_idx[i][j], SB)
                    return 0

                # masked tiles first so their (slow, GPSIMD) mask ops get the
                # whole block to complete (their PVs are deferred to the end);
                # within them, smallest trim first so the first-emitted PV
                # covers the whole block (it clears the psum accumulation).
                ilist.sort(key=lambda i: (kind[i][j] == "plain", trim(i)))
                for i in ilist:
                    s0 = trim(i)
                    # fp32r score matmuls drop to 1/4 rate below 256 moving
                    s0_sc = min(s0, SB - 256)
                    tsl = slice(b * SEQ + i * P, b * SEQ + (i + 1) * P)
                    sc = scps.tile([P, 2, SB], dt.float32, tag="sc", name="sc")
                    for h in range(2):
                        hs = slice(h * 64, h * 64 + 64)
                        nc.tensor.matmul(
                            sc[:, h, s0_sc:],
                            lhsT=kh_sb[hs, tsl],
                            rhs=qh_sb[hs, ssl][:, s0_sc:],
                            start=True,
                            stop=True,
                        )
                    e = expp.tile([P, 2, SB], dt.bfloat16, tag="e", name="e")
                    nc.scalar.activation(
                        e[:, :, s0:],
                        sc[:, :, s0:],
                        mybir.ActivationFunctionType.Exp,
                        scale=1.0 / float(np.sqrt(DEPTH)),
                    )
                    if kind[i][j] == "affine":
                        # zero e[t', h, s'] where s' < c + t' (GPSIMD)
                        nc.gpsimd.affine_select(
                            out=e[:, :, s0:],
                            in_=e[:, :, s0:],
                            pattern=[[0, 2], [1, SB - s0]],
                            compare_op=mybir.AluOpType.is_ge,
                            fill=0.0,
                            base=s0 - mix_idx[i][j],
                            channel_multiplier=-1,
                        )
                        deferred.append((i, e, s0))
                    elif kind[i][j] == "mixed":
                        u = mix_idx[i][j]
                        nc.vector.tensor_tensor(
                            e,
                            e,
                            mask_sb[:, u, None, :].to_broadcast(e.shape),
                            mybir.AluOpType.mult,
                        )
                        deferred.append((i, e, 0))
                    else:
                        emit_pv(i, e, 0)
                    # interleave cross-phase work, paced so a DMA-heavy
                    # projection unit (~2.9us of DMA) gets ~3 score
                    # iterations (~1us each) of headroom. Single ordered
                    # queue: emission order must respect dataflow.
                    budget[0] += 1
                    while filler and budget[0] >= filler[0][1]:
                        unit, cost, _ = filler.popleft()
                        budget[0] -= cost
                        unit()
                for i, e, s0 in deferred:
                    emit_pv(i, e, s0)
                # normalize: l must land on the data's partitions; engines
                # can't cross partitions, so bounce through an SBUF DMA (on
                # the gpsimd queue to keep the sync sequencer free).
                ltmp = linp.tile([P, SB], dt.float32, tag="ltmp", name="ltmp")
                nc.vector.tensor_copy(ltmp[64:128, :], pv[0][64:128, :])
                nc.vector.tensor_copy(ltmp[0:64, :], pv[1][0:64, :])
                lin = linp.tile([P, SB], dt.float32, tag="lin", name="lin")
                nc.gpsimd.dma_start(lin[0:64, :], ltmp[64:128, :])
                nc.gpsimd.dma_start(lin[64:128, :], ltmp[0:64, :])
                nc.vector.reciprocal(lin, lin)
                nc.vector.tensor_tensor(
                    ao_sb[0:64, ssl], pv[0][0:64, :], lin[0:64, :],
                    mybir.AluOpType.mult,
                )
                nc.vector.tensor_tensor(
                    ao_sb[64:128, ssl], pv[1][64:128, :], lin[64:128, :],
                    mybir.AluOpType.mult,
                )

            def outproj_nb(nb, dgroup):
                """Output projection for n-block nb, d-tiles [2*dgroup, +2)."""
                nsl = slice(nb * SB, (nb + 1) * SB)
                ost = outst.tile([P, 2, SB], dt.bfloat16, tag="ost", name="ost")
                for u in range(2):
                    dtile = 2 * dgroup + u
                    ps = mmps.tile([P, SB], dt.float32, tag="ps", name="po")
                    nc.tensor.matmul(
                        ps,
                        lhsT=wo_sb[:, dtile, :],
                        rhs=ao_sb[:, nsl],
                        start=True,
                        stop=True,
                    )
                    nc.vector.tensor_copy(ost[:, u, :], ps)
                nc.sync.dma_start(
                    outT[:, nsl]
                    .rearrange("(ct p) n -> p ct n", p=P)[:, 2 * dgroup : 2 * dgroup + 2, :],
                    ost,
                )

            def proj_units(b, m):
                nb = b * (N_NB // 2) + m
                return [
                    lambda: project_nb(kT, wk_sb, kh_sb, "k", nb),
                    lambda: project_nb(qT, wq_sb, qh_sb, "q", nb),
                    lambda: project_nb(vT, wv_sb, vhT_sb, "v", nb),
                ]

            # driver: only block (0,0)'s projections run serially; later
            # projections, v transposes and output projections of finished
            # blocks are interleaved into the attention stream.
            from collections import deque

            filler = deque()
            budget = [0]

            def push_block_units(b, m):
                for u in proj_units(b, m):
                    filler.append((u, 3, (b, m)))
                filler.append((lambda x=m, y=b: transpose_vh(y, 0, x), 1, (b, m)))
                filler.append((lambda x=m, y=b: transpose_vh(y, 1, x), 1, (b, m)))

            for u in proj_units(0, 0):
                u()
            transpose_vh(0, 0, 0)
            transpose_vh(0, 1, 0)
            for m in range(1, N_SB):
                push_block_units(0, m)
            for m in range(N_SB):
                push_block_units(1, m)
            for b in range(BATCH):
                for j in range(N_SB):
                    attention_block(b, j)
                    for dg in range(N_CT // 2):
                        filler.append(
                            (
                                lambda x=b * (N_NB // 2) + j, g=dg: outproj_nb(x, g),
                                1,
                                None,
                            )
                        )
            while filler:
                filler.popleft()[0]()

    nc.compile()
    return nc


def kernel(v, k, q, mask, Wq, bq, Wk, bk, Wv, bv, Wo, bo, trace=False):
    global LAST_RESULTS
    from concourse.bass_utils import run_bass_kernel_spmd

    in_np = ml_dtypes.bfloat16 if IN_BF16 else np.float32

    def prep_T(x):  # [B, S, D] -> [D, NTOK] in input dtype
        return np.ascontiguousarray(
            np.asarray(x, dtype=np.float32).reshape(NTOK, D_MODEL).T
        ).astype(in_np)

    qT = prep_T(q)
    kT = prep_T(k)
    vT = prep_T(v)

    kind, mix_idx, patterns = _mask_structure(np.asarray(mask, dtype=np.float32))
    maskt = (
        np.ascontiguousarray(np.stack(patterns)).astype(ml_dtypes.bfloat16)
        if patterns
        else None
    )

    has_bias = bool(
        np.any(np.asarray(bq)) or np.any(np.asarray(bk)) or np.any(np.asarray(bv))
    )

    nc = _build_nc(kind, mix_idx, len(patterns), IN_BF16, has_bias)

    in_maps = []
    for core in range(N_CORES):
        fsl = slice(core * FW, (core + 1) * FW)
        m = {
            "qT": qT,
            "kT": kT,
            "vT": vT,
            "wqT": np.ascontiguousarray(np.asarray(Wq)[fsl].T).astype(in_np),
            "wkT": np.ascontiguousarray(np.asarray(Wk)[fsl].T).astype(in_np),
            "wvT": np.ascontiguousarray(np.asarray(Wv)[fsl].T).astype(in_np),
            "woT": np.ascontiguousarray(np.asarray(Wo)[:, fsl].T).astype(np.float32),
            "identT": np.concatenate(
                [np.eye(64, dtype=np.float32)] * 2, axis=0
            ).astype(ml_dtypes.bfloat16),
        }
        if maskt is not None:
            m["maskt"] = maskt
        if has_bias:
            m["bq"] = np.asarray(bq, np.float32)[fsl].reshape(FW, 1)
            m["bk"] = np.asarray(bk, np.float32)[fsl].reshape(FW, 1)
            m["bv"] = np.asarray(bv, np.float32)[fsl].reshape(FW, 1)
        in_maps.append(m)

    import time as _time

    global LAST_EXEC_WALL
    _t0 = _time.time()
    res = run_bass_kernel_spmd(
        nc, in_maps, core_ids=list(range(N_CORES)), trace=trace
    )
    LAST_EXEC_WALL = _time.time() - _t0
    LAST_RESULTS = res

    acc = np.zeros((D_MODEL, NTOK), dtype=np.float32)
    for r in res.results:
        acc += r["outT"].astype(np.float32)
    acc += np.asarray(bo, np.float32)[:, None]
    return np.ascontiguousarray(acc.T).reshape(BATCH, SEQ, D_MODEL)


# revision 55
# speedup vs baseline: 1.0226x; 1.0226x over previous
"""Multi-head attention (B=2, S=2048, D=1024, H=16) on 8 Trainium2 cores.

Sharding: tensor-parallel over heads — each core owns 2 heads (a 128-feature
slice) for both batches.  Per core:
  - QKV projections for its feature slice (full tokens), transposed layout
  - causal attention for its 4 (batch, head) pairs with block-skipping
  - partial output projection (contraction over its 128 features)
Host: transposes/prepares inputs, sums the 8 partial outputs, adds bo.

On-chip pipeline: inputs/weights arrive in IN_DT (bf16 default), q/k
projections are stored float32r and the score matmuls run float32r (full PE
rate at moving-dim >= 256, near-fp32 precision); the v path and attention
probabilities are bf16 (PV runs bf16 at full rate).  v is transposed to
token-major with small PE transposes.  The causal mask is applied with
GPSIMD affine_select on the exp output — masked tiles' PV matmuls are
deferred to the end of each block so the in-order PE never waits on the
(slow) mask op, and fully-masked column prefixes of diagonal tiles are
skipped in scores/exp/PV.  Cross-phase work (next-batch projections,
finished-block output projections) is interleaved into the attention
instruction stream with DMA-rate pacing.
"""

import os

import numpy as np
import ml_dtypes

D_MODEL = 1024
NUM_HEADS = 16
DEPTH = 64
BATCH = 2
SEQ = 2048
NTOK = BATCH * SEQ  # 4096
N_CORES = 8
FW = 128  # features per core (2 heads x 64)
P = 128
SB = 512  # s-block width
N_SB = SEQ // SB  # 4 s-blocks per batch
N_TT = SEQ // P  # 16 t-tiles per batch
N_NB = NTOK // SB  # 8 n-blocks over all tokens
N_CT = D_MODEL // P  # 8 contraction tiles

# bf16 inputs halve DMA traffic; fp32 inputs maximize accuracy.
IN_BF16 = os.environ.get("MHA_IN_BF16", "1") == "1"

LAST_RESULTS = None  # BassKernelResults from the most recent kernel() call
LAST_EXEC_WALL = None  # wall seconds of the run_bass_kernel_spmd call


def _mask_structure(mask_np):
    """Classify each (t-tile, s-block) of the [S, S] mask (1.0 = disallowed).

    Returns (kind, mix_idx, patterns): kind[i][j] in
    {"skip", "plain", "affine", "mixed"}; for "affine", mix_idx[i][j] is the
    offset c of keep = (s >= c + t); for "mixed" it indexes into patterns
    (list of [P, SB] keep-masks).  mask rows = query s, cols = key t;
    scoresT is [t, s] so we transpose.
    """
    maskT = np.ascontiguousarray(mask_np.reshape(SEQ, SEQ).T)
    kind = [[None] * N_SB for _ in range(N_TT)]
    mix_idx = [[None] * N_SB for _ in range(N_TT)]
    patterns = []
    pat_key = {}
    s_idx = np.arange(SB)[None, :]
    t_idx = np.arange(P)[:, None]
    for i in range(N_TT):
        for j in range(N_SB):
            sub = maskT[i * P : (i + 1) * P, j * SB : (j + 1) * SB]
            if np.all(sub >= 0.5):
                kind[i][j] = "skip"
                continue
            if np.all(sub < 0.5):
                kind[i][j] = "plain"
                continue
            keep = (sub < 0.5).astype(np.float32)
            first_one = np.argmax(keep, axis=1)
            c = int(first_one[0])
            if np.array_equal(keep, (s_idx >= c + t_idx).astype(np.float32)):
                kind[i][j] = "affine"
                mix_idx[i][j] = c
                continue
            kind[i][j] = "mixed"
            key = keep.tobytes()
            if key not in pat_key:
                pat_key[key] = len(patterns)
                patterns.append(keep)
            mix_idx[i][j] = pat_key[key]
    return kind, mix_idx, patterns


def _build_nc(kind, mix_idx, n_patterns, in_bf16, has_bias):
    import concourse.tile as tile
    import concourse.mybir as mybir
    from concourse import bacc

    dt = mybir.dt
    # walrus requires every producer feeding an fp32r matmul to emit
    # fp32r-typed (rounded) values, so fp32-mode inputs and the q/k/ao/wo
    # activations are declared float32r outright (same bytes as fp32).
    IN_DT = dt.bfloat16 if in_bf16 else dt.float32r

    nc = bacc.Bacc(None, target_bir_lowering=False)

    qT = nc.dram_tensor("qT", [D_MODEL, NTOK], IN_DT, kind="ExternalInput")
    kT = nc.dram_tensor("kT", [D_MODEL, NTOK], IN_DT, kind="ExternalInput")
    vT = nc.dram_tensor("vT", [D_MODEL, NTOK], IN_DT, kind="ExternalInput")
    wq = nc.dram_tensor("wqT", [D_MODEL, FW], IN_DT, kind="ExternalInput")
    wk = nc.dram_tensor("wkT", [D_MODEL, FW], IN_DT, kind="ExternalInput")
    wv = nc.dram_tensor("wvT", [D_MODEL, FW], IN_DT, kind="ExternalInput")
    wo = nc.dram_tensor("woT", [FW, D_MODEL], dt.float32r, kind="ExternalInput")
    maskt = None
    if n_patterns:
        maskt = nc.dram_tensor(
            "maskt", [n_patterns, P, SB], dt.bfloat16, kind="ExternalInput"
        )
    bq = bk = bv = None
    if has_bias:
        bq = nc.dram_tensor("bq", [FW, 1], dt.float32, kind="ExternalInput")
        bk = nc.dram_tensor("bk", [FW, 1], dt.float32, kind="ExternalInput")
        bv = nc.dram_tensor("bv", [FW, 1], dt.float32, kind="ExternalInput")
    identT = nc.dram_tensor("identT", [P, 64], dt.bfloat16, kind="ExternalInput")
    outT = nc.dram_tensor("outT", [D_MODEL, NTOK], dt.bfloat16, kind="ExternalOutput")

    with tile.TileContext(nc) as tc:
        with (
            tc.tile_pool(name="const", bufs=1) as const,
            tc.tile_pool(name="big", bufs=1) as big,
            tc.tile_pool(name="stageA", bufs=6) as stA,
            tc.tile_pool(name="expp", bufs=6) as expp,
            tc.tile_pool(name="linp", bufs=2) as linp,
            tc.tile_pool(name="outst", bufs=6) as outst,
            # PSUM: sc 2x2 banks + pv 2x1 + shared proj/outproj 2 = 8 banks
            tc.tile_pool(name="mmps", bufs=2, space="PSUM") as mmps,
            tc.tile_pool(name="scps", bufs=2, space="PSUM") as scps,
            tc.tile_pool(name="pvps", bufs=1, space="PSUM") as pvps,
        ):
            # ---- constants ----
            wq_sb = const.tile([P, N_CT, FW], IN_DT)
            wk_sb = const.tile([P, N_CT, FW], IN_DT)
            wv_sb = const.tile([P, N_CT, FW], IN_DT)
            nc.sync.dma_start(wq_sb, wq.rearrange("(ct p) f -> p ct f", p=P))
            nc.sync.dma_start(wk_sb, wk.rearrange("(ct p) f -> p ct f", p=P))
            nc.sync.dma_start(wv_sb, wv.rearrange("(ct p) f -> p ct f", p=P))
            wo_sb = const.tile([P, N_CT, P], dt.float32r)
            nc.sync.dma_start(wo_sb, wo.rearrange("f (dt p) -> f dt p", p=P))
            mask_sb = None
            if n_patterns:
                mask_sb = const.tile([P, n_patterns, SB], dt.bfloat16)
                nc.sync.dma_start(mask_sb, maskt.rearrange("m p s -> p m s"))
            # [128, 64] = eye(64) stacked twice (bf16): PE-transpose identity;
            # the h=1 slice needs base partition 64.
            ident = const.tile([P, 64], dt.bfloat16)
            nc.sync.dma_start(ident, identT[:, :])
            bias_sb = {}
            if has_bias:
                for name, t in (("q", bq), ("k", bk), ("v", bv)):
                    bias_sb[name] = const.tile([P, 1], dt.float32)
                    nc.sync.dma_start(bias_sb[name], t[:, :])

            # ---- persistent activations ----
            qh_sb = big.tile([P, NTOK], dt.float32r)
            kh_sb = big.tile([P, NTOK], dt.float32r)
            ao_sb = big.tile([P, NTOK], dt.float32r)
            vhT_sb = big.tile([P, NTOK], dt.bfloat16)
            # per (b, h): [t', t-tile, 128] bf16; one 64-col half holds v
            # (written by PE transposes), the other is 1.0 so the PV
            # matmul also produces the softmax row-sums l:
            #   h0: lhsT = [v | 1] -> psum [data(0:64); l(64:128)]
            #   h1: lhsT = [1 | v] -> psum [l(0:64); data(64:128)]
            vh_sb = [
                big.tile([P, N_TT, P], dt.bfloat16, name=f"vh{pair}")
                for pair in range(4)
            ]
            for pair in range(4):
                h = pair % 2
                ones_sl = slice(64, 128) if h == 0 else slice(0, 64)
                nc.vector.memset(vh_sb[pair][:, :, ones_sl], 1.0)

            def proj_copyback(dst_ap, ps, bias_tile):
                if bias_tile is not None:
                    nc.vector.tensor_tensor(
                        dst_ap, ps, bias_tile.to_broadcast(ps.shape),
                        mybir.AluOpType.add,
                    )
                else:
                    nc.vector.tensor_copy(dst_ap, ps)

            def project_nb(src, w_sb, dst, bn, nb):
                """Project one 512-token block of one of q/k/vT."""
                st = stA.tile([P, N_CT, SB], IN_DT, tag="st", name="st")
                nc.sync.dma_start(
                    st,
                    src[:, nb * SB : (nb + 1) * SB].rearrange(
                        "(ct p) n -> p ct n", p=P
                    ),
                )
                ps = mmps.tile([P, SB], dt.float32, tag="ps", name="ps")
                for ct in range(N_CT):
                    nc.tensor.matmul(
                        ps,
                        lhsT=w_sb[:, ct, :],
                        rhs=st[:, ct, :],
                        start=(ct == 0),
                        stop=(ct == N_CT - 1),
                    )
                proj_copyback(
                    dst[:, nb * SB : (nb + 1) * SB],
                    ps,
                    bias_sb.get(bn) if has_bias else None,
                )

            def transpose_vh(b, h, m):
                """One 512-token chunk of vhT [64, SB] -> vh natural
                [128, 4, 64] via PE transposes (the XBAR DMA transpose
                serializes against every in-flight DMA on mode switch)."""
                pair = 2 * b + h
                data_sl = slice(0, 64) if h == 0 else slice(64, 128)
                hsl = slice(h * 64, h * 64 + 64)
                for tt in range(4 * m, 4 * m + 4):
                    tok0 = b * SEQ + tt * P
                    pst = mmps.tile([P, 64], dt.bfloat16, tag="ps", name="tp")
                    nc.tensor.transpose(
                        pst, vhT_sb[hsl, tok0 : tok0 + P], ident[hsl, :]
                    )
                    nc.vector.tensor_copy(vh_sb[pair][:, tt, data_sl], pst)

            def flush_filler_upto(b, j):
                """Emit all queued units that attention block (b, j) depends
                on (its own batch's projections/transposes up to block j)."""
                while filler and (
                    filler[0][2] is None or filler[0][2] <= (b, j)
                ):
                    unit, cost, _ = filler.popleft()
                    unit()

            def attention_block(b, j):
                flush_filler_upto(b, j)
                ilist = [i for i in range(N_TT) if kind[i][j] != "skip"]
                assert ilist, "fully-masked s-block unsupported"

                pv = [
                    pvps.tile([P, SB], dt.float32, tag=f"pv{h}", name=f"pv{h}")
                    for h in range(2)
                ]
                ssl = slice(b * SEQ + j * SB, b * SEQ + (j + 1) * SB)
                n_pv = len(ilist)
                pv_emitted = 0
                deferred = []

                def emit_pv(i, e, s0):
                    nonlocal pv_emitted
                    for h in range(2):
                        nc.tensor.matmul(
                            pv[h][:, s0:],
                            lhsT=vh_sb[2 * b + h][:, i, :],
                            rhs=e[:, h, s0:],
                            start=(pv_emitted == 0),
                            stop=(pv_emitted == n_pv - 1),
                        )
                    pv_emitted += 1

                # an affine tile with offset c has columns s' < c fully
                # masked: skip them in scores/exp/PV. Only safe if the
                # first-emitted PV (which clears the psum accumulation
                # group) covers the whole block.
                can_trim = any(
                    kind[i][j] == "plain"
                    or (kind[i][j] == "affine" and mix_idx[i][j] == 0)
                    for i in ilist
                )

                def trim(i):
                    if can_trim and kind[i][j] == "affine":
                        return min(mix_idx[i][j], SB)
                    return 0

                # masked tiles first so their (slow, GPSIMD) mask ops get the
                # whole block to complete (their PVs are deferred to the end);
                # within them, smallest trim first so the first-emitted PV
                # covers the whole block (it clears the psum accumulation).
                ilist.sort(key=lambda i: (kind[i][j] == "plain", trim(i)))
                for i in ilist:
                    s0 = trim(i)
                    # fp32r score matmuls drop to 1/4 rate below 256 moving
                    s0_sc = min(s0, SB - 256)
                    tsl = slice(b * SEQ + i * P, b * SEQ + (i + 1) * P)
                    sc = scps.tile([P, 2, SB], dt.float32, tag="sc", name="sc")
                    for h in range(2):
                        hs = slice(h * 64, h * 64 + 64)
                        nc.tensor.matmul(
                            sc[:, h, s0_sc:],
                            lhsT=kh_sb[hs, tsl],
                            rhs=qh_sb[hs, ssl][:, s0_sc:],
                            start=True,
                            stop=True,
                        )
                    e = expp.tile([P, 2, SB], dt.bfloat16, tag="e", name="e")
                    nc.scalar.activation(
                        e[:, :, s0:],
                        sc[:, :, s0:],
                        mybir.ActivationFunctionType.Exp,
                        scale=1.0 / float(np.sqrt(DEPTH)),
                    )
                    if kind[i][j] == "affine":
                        # zero e[t', h, s'] where s' < c + t' (GPSIMD)
                        nc.gpsimd.affine_select(
                            out=e[:, :, s0:],
                            in_=e[:, :, s0:],
                            pattern=[[0, 2], [1, SB - s0]],
                            compare_op=mybir.AluOpType.is_ge,
                            fill=0.0,
                            base=s0 - mix_idx[i][j],
                            channel_multiplier=-1,
                        )
                        deferred.append((i, e, s0))
                    elif kind[i][j] == "mixed":
                        u = mix_idx[i][j]
                        nc.vector.tensor_tensor(
                            e,
                            e,
                            mask_sb[:, u, None, :].to_broadcast(e.shape),
                            mybir.AluOpType.mult,
                        )
                        deferred.append((i, e, 0))
                    else:
                        emit_pv(i, e, 0)
                    # interleave cross-phase work, paced so a DMA-heavy
                    # projection unit (~2.9us of DMA) gets ~3 score
                    # iterations (~1us each) of headroom. Single ordered
                    # queue: emission order must respect dataflow.
                    budget[0] += 1
                    while filler and budget[0] >= filler[0][1]:
                        unit, cost, _ = filler.popleft()
                        budget[0] -= cost
                        unit()
                for i, e, s0 in deferred:
                    emit_pv(i, e, s0)
                # normalize: l must land on the data's partitions; engines
                # can't cross partitions, so bounce through an SBUF DMA (on
                # the gpsimd queue to keep the sync sequencer free).
                ltmp = linp.tile([P, SB], dt.float32, tag="ltmp", name="ltmp")
                nc.vector.tensor_copy(ltmp[64:128, :], pv[0][64:128, :])
                nc.vector.tensor_copy(ltmp[0:64, :], pv[1][0:64, :])
                lin = linp.tile([P, SB], dt.float32, tag="lin", name="lin")
                nc.gpsimd.dma_start(lin[0:64, :], ltmp[64:128, :])
                nc.gpsimd.dma_start(lin[64:128, :], ltmp[0:64, :])
                nc.vector.reciprocal(lin, lin)
                nc.vector.tensor_tensor(
                    ao_sb[0:64, ssl], pv[0][0:64, :], lin[0:64, :],
                    mybir.AluOpType.mult,
                )
                nc.vector.tensor_tensor(
                    ao_sb[64:128, ssl], pv[1][64:128, :], lin[64:128, :],
                    mybir.AluOpType.mult,
                )

            def outproj_nb(nb, dgroup):
                """Output projection for n-block nb, d-tiles [2*dgroup, +2)."""
                nsl = slice(nb * SB, (nb + 1) * SB)
                ost = outst.tile([P, 2, SB], dt.bfloat16, tag="ost", name="ost")
                for u in range(2):
                    dtile = 2 * dgroup + u
                    ps = mmps.tile([P, SB], dt.float32, tag="ps", name="po")
                    nc.tensor.matmul(
                        ps,
                        lhsT=wo_sb[:, dtile, :],
                        rhs=ao_sb[:, nsl],
                        start=True,
                        stop=True,
                    )
                    nc.vector.tensor_copy(ost[:, u, :], ps)
                nc.sync.dma_start(
                    outT[:, nsl]
                    .rearrange("(ct p) n -> p ct n", p=P)[:, 2 * dgroup : 2 * dgroup + 2, :],
                    ost,
                )

            def proj_units(b, m):
                nb = b * (N_NB // 2) + m
                return [
                    lambda: project_nb(kT, wk_sb, kh_sb, "k", nb),
                    lambda: project_nb(qT, wq_sb, qh_sb, "q", nb),
                    lambda: project_nb(vT, wv_sb, vhT_sb, "v", nb),
                ]

            # driver: only block (0,0)'s projections run serially; later
            # projections, v transposes and output projections of finished
            # blocks are interleaved into the attention stream.
            from collections import deque

            filler = deque()
            budget = [0]

            def push_block_units(b, m):
                for u in proj_units(b, m):
                    filler.append((u, 3, (b, m)))
                filler.append((lambda x=m, y=b: transpose_vh(y, 0, x), 1, (b, m)))
                filler.append((lambda x=m, y=b: transpose_vh(y, 1, x), 1, (b, m)))

            for u in proj_units(0, 0):
                u()
            transpose_vh(0, 0, 0)
            transpose_vh(0, 1, 0)
            for m in range(1, N_SB):
                push_block_units(0, m)
            for m in range(N_SB):
                push_block_units(1, m)
            for b in range(BATCH):
                for j in range(N_SB):
                    attention_block(b, j)
                    for dg in range(N_CT // 2):
                        filler.append(
                            (
                                lambda x=b * (N_NB // 2) + j, g=dg: outproj_nb(x, g),
                                1,
                                None,
                            )
                        )
            while filler:
                filler.popleft()[0]()

    nc.compile()
    return nc


def kernel(v, k, q, mask, Wq, bq, Wk, bk, Wv, bv, Wo, bo, trace=False):
    global LAST_RESULTS
    from concourse.bass_utils import run_bass_kernel_spmd

    in_np = ml_dtypes.bfloat16 if IN_BF16 else np.float32

    def prep_T(x):  # [B, S, D] -> [D, NTOK] in input dtype
        return np.ascontiguousarray(
            np.asarray(x, dtype=np.float32).reshape(NTOK, D_MODEL).T
        ).astype(in_np)

    qT = prep_T(q)
    kT = prep_T(k)
    vT = prep_T(v)

    kind, mix_idx, patterns = _mask_structure(np.asarray(mask, dtype=np.float32))
    maskt = (
        np.ascontiguousarray(np.stack(patterns)).astype(ml_dtypes.bfloat16)
        if patterns
        else None
    )

    has_bias = bool(
        np.any(np.asarray(bq)) or np.any(np.asarray(bk)) or np.any(np.asarray(bv))
    )

    nc = _build_nc(kind, mix_idx, len(patterns), IN_BF16, has_bias)

    in_maps = []
    for core in range(N_CORES):
        fsl = slice(core * FW, (core + 1) * FW)
        m = {
            "qT": qT,
            "kT": kT,
            "vT": vT,
            "wqT": np.ascontiguousarray(np.asarray(Wq)[fsl].T).astype(in_np),
            "wkT": np.ascontiguousarray(np.asarray(Wk)[fsl].T).astype(in_np),
            "wvT": np.ascontiguousarray(np.asarray(Wv)[fsl].T).astype(in_np),
            "woT": np.ascontiguousarray(np.asarray(Wo)[:, fsl].T).astype(np.float32),
            "identT": np.concatenate(
                [np.eye(64, dtype=np.float32)] * 2, axis=0
            ).astype(ml_dtypes.bfloat16),
        }
        if maskt is not None:
            m["maskt"] = maskt
        if has_bias:
            m["bq"] = np.asarray(bq, np.float32)[fsl].reshape(FW, 1)
            m["bk"] = np.asarray(bk, np.float32)[fsl].reshape(FW, 1)
            m["bv"] = np.asarray(bv, np.float32)[fsl].reshape(FW, 1)
        in_maps.append(m)

    import time as _time

    global LAST_EXEC_WALL
    _t0 = _time.time()
    res = run_bass_kernel_spmd(
        nc, in_maps, core_ids=list(range(N_CORES)), trace=trace
    )
    LAST_EXEC_WALL = _time.time() - _t0
    LAST_RESULTS = res

    acc = np.zeros((D_MODEL, NTOK), dtype=np.float32)
    for r in res.results:
        acc += r["outT"].astype(np.float32)
    acc += np.asarray(bo, np.float32)[:, None]
    return np.ascontiguousarray(acc.T).reshape(BATCH, SEQ, D_MODEL)
